# revision 3
# baseline (speedup 1.0000x reference)
"""Trainium2 Bass kernel for nn_AttackLoss (nms_detection).

Computes, for O=2048 ground-truth boxes vs D=8732 detections:
    best[o]  = max IoU over same-label detections
    loss     = sum(has_match * (1 - best)) / sum(has_match)

Sharding: objects are split across 8 NeuronCores (256 per core). Each core
holds all detections laid out on SBUF partitions (128 x 69 tiles) and its
object slab broadcast along the free axis; it reduces to two partial scalars
(sum of matched (1-best), match count). The host sums the 8 partials.
"""

import os
import tempfile
from contextlib import ExitStack

import numpy as np

import concourse.bacc as bacc
import concourse.bass as bass
import concourse.mybir as mybir
import concourse.tile as tile
from concourse.bass_isa import ReduceOp
from concourse.bass_utils import run_bass_kernel_spmd

F32 = mybir.dt.float32
OP = mybir.AluOpType
AX = mybir.AxisListType

N_CORES = 8
N_DET = 8732
N_OBJ = 2048
OBJ_PER_CORE = N_OBJ // N_CORES  # 256
T_DET = 69                        # ceil(8732/128)
DET_PAD = 128 * T_DET             # 8832


def _build_dense():
    """Dense kernel: all dets (on partitions) x this core's objects (free)."""
    nc = bacc.Bacc("TRN2", target_bir_lowering=False, debug=False,
                   num_devices=N_CORES)
    F = OBJ_PER_CORE

    detp_d = nc.dram_tensor("detp", [128, 5, T_DET], F32, kind="ExternalInput")
    objr_d = nc.dram_tensor("objr", [5, F], F32, kind="ExternalInput")
    part_d = nc.dram_tensor("partial", [1, 2], F32, kind="ExternalOutput")

    with tile.TileContext(nc) as tc, ExitStack() as ctx:
        cpool = ctx.enter_context(tc.tile_pool(name="const", bufs=1))
        wpool = ctx.enter_context(tc.tile_pool(name="work", bufs=3))

        detp = cpool.tile([128, 5, T_DET], F32)
        nc.sync.dma_start(detp[:], detp_d[:])
        # broadcast object rows across partitions
        names = ["ox1", "oy1", "ox2", "oy2", "olab"]
        ob = {}
        for i, nm in enumerate(names):
            row = cpool.tile([1, F], F32, tag=f"r_{nm}")
            nc.sync.dma_start(row[:], objr_d[i:i + 1, :])
            t = cpool.tile([128, F], F32, tag=f"b_{nm}")
            nc.gpsimd.partition_broadcast(t[:], row[:], channels=128)
            ob[nm] = t

        # object areas [128, F]
        aob = cpool.tile([128, F], F32)
        wob = wpool.tile([128, F], F32, tag="wob")
        nc.vector.tensor_tensor(wob[:], ob["ox2"][:], ob["ox1"][:], OP.subtract)
        hob = wpool.tile([128, F], F32, tag="hob")
        nc.vector.tensor_tensor(hob[:], ob["oy2"][:], ob["oy1"][:], OP.subtract)
        nc.vector.tensor_tensor(aob[:], wob[:], hob[:], OP.mult)

        # det areas [128, T]
        ad = cpool.tile([128, T_DET], F32)
        wd = wpool.tile([128, T_DET], F32, tag="wd")
        nc.vector.tensor_tensor(wd[:], detp[:, 2, :], detp[:, 0, :], OP.subtract)
        hd = wpool.tile([128, T_DET], F32, tag="hd")
        nc.vector.tensor_tensor(hd[:], detp[:, 3, :], detp[:, 1, :], OP.subtract)
        nc.vector.tensor_tensor(ad[:], wd[:], hd[:], OP.mult)

        bmax = cpool.tile([128, F], F32)
        nc.vector.memset(bmax[:], 0.0)
        hm = cpool.tile([128, F], F32)
        nc.vector.memset(hm[:], 0.0)

        for t in range(T_DET):
            dx1 = detp[:, 0, t:t + 1]
            dy1 = detp[:, 1, t:t + 1]
            dx2 = detp[:, 2, t:t + 1]
            dy2 = detp[:, 3, t:t + 1]
            dlab = detp[:, 4, t:t + 1]
            adt = ad[:, t:t + 1]

            mnx = wpool.tile([128, F], F32, tag="mnx")
            nc.vector.tensor_scalar(mnx[:], ob["ox2"][:], dx2, None, op0=OP.min)
            mxx = wpool.tile([128, F], F32, tag="mxx")
            nc.vector.tensor_scalar(mxx[:], ob["ox1"][:], dx1, None, op0=OP.max)
            wx = wpool.tile([128, F], F32, tag="wx")
            nc.vector.tensor_tensor(wx[:], mnx[:], mxx[:], OP.subtract)
            wxr = wpool.tile([128, F], F32, tag="wxr")
            nc.vector.tensor_scalar(wxr[:], wx[:], 0.0, None, op0=OP.max)

            mny = wpool.tile([128, F], F32, tag="mny")
            nc.vector.tensor_scalar(mny[:], ob["oy2"][:], dy2, None, op0=OP.min)
            mxy = wpool.tile([128, F], F32, tag="mxy")
            nc.vector.tensor_scalar(mxy[:], ob["oy1"][:], dy1, None, op0=OP.max)
            wy = wpool.tile([128, F], F32, tag="wy")
            nc.vector.tensor_tensor(wy[:], mny[:], mxy[:], OP.subtract)
            wyr = wpool.tile([128, F], F32, tag="wyr")
            nc.vector.tensor_scalar(wyr[:], wy[:], 0.0, None, op0=OP.max)

            inter = wpool.tile([128, F], F32, tag="inter")
            nc.vector.tensor_tensor(inter[:], wxr[:], wyr[:], OP.mult)
            sab = wpool.tile([128, F], F32, tag="sab")
            nc.vector.tensor_scalar(sab[:], aob[:], adt, None, op0=OP.add)
            denom = wpool.tile([128, F], F32, tag="denom")
            nc.vector.tensor_tensor(denom[:], sab[:], inter[:], OP.subtract)
            rec = wpool.tile([128, F], F32, tag="rec")
            nc.vector.reciprocal(rec[:], denom[:])
            iou = wpool.tile([128, F], F32, tag="iou")
            nc.vector.tensor_tensor(iou[:], inter[:], rec[:], OP.mult)

            eq = wpool.tile([128, F], F32, tag="eq")
            nc.vector.tensor_scalar(eq[:], ob["olab"][:], dlab, None,
                                    op0=OP.is_equal)
            miou = wpool.tile([128, F], F32, tag="miou")
            nc.vector.tensor_tensor(miou[:], iou[:], eq[:], OP.mult)

            nc.vector.tensor_tensor(bmax[:], bmax[:], miou[:], OP.max)
            nc.vector.tensor_tensor(hm[:], hm[:], eq[:], OP.max)

        bred = cpool.tile([128, F], F32)
        nc.gpsimd.partition_all_reduce(bred[:], bmax[:], 128, ReduceOp.max)
        hred = cpool.tile([128, F], F32)
        nc.gpsimd.partition_all_reduce(hred[:], hm[:], 128, ReduceOp.max)

        c1 = wpool.tile([1, F], F32, tag="c1")
        nc.vector.tensor_scalar(c1[:], bred[0:1, :], -1.0, 1.0,
                                op0=OP.mult, op1=OP.add)
        c2 = wpool.tile([1, F], F32, tag="c2")
        nc.vector.tensor_tensor(c2[:], c1[:], hred[0:1, :], OP.mult)

        outt = wpool.tile([1, 2], F32, tag="outt")
        nc.vector.tensor_reduce(outt[:, 0:1], c2[:], AX.X, OP.add)
        nc.vector.tensor_reduce(outt[:, 1:2], hred[0:1, :], AX.X, OP.add)
        nc.sync.dma_start(part_d[:], outt[:])

    nc.compile()
    return nc


def _prep_dense_inputs(det_boxes, det_labels, boxes, labels):
    """Build per-core in_maps for the dense kernel (numpy only)."""
    det = np.full((DET_PAD, 5), -5.0, dtype=np.float32)
    det[:N_DET, 0:4] = det_boxes.astype(np.float32)
    det[:N_DET, 4] = det_labels.astype(np.float32)
    det[N_DET:, 4] = -1.0
    # [DET_PAD, 5] -> [T, 128, 5] -> [128, 5, T]
    detp = np.ascontiguousarray(
        det.reshape(T_DET, 128, 5).transpose(1, 2, 0))

    in_maps = []
    for c in range(N_CORES):
        sl = slice(c * OBJ_PER_CORE, (c + 1) * OBJ_PER_CORE)
        objr = np.empty((5, OBJ_PER_CORE), dtype=np.float32)
        objr[0:4, :] = boxes[sl].astype(np.float32).T
        objr[4, :] = labels[sl].astype(np.float32)
        in_maps.append({"detp": detp, "objr": objr})
    return in_maps


_CACHE = {}


def _get_dense():
    if "dense" not in _CACHE:
        _CACHE["dense"] = _build_dense()
    return _CACHE["dense"]


def kernel(det_boxes, det_scores, det_labels, boxes, labels):
    nc = _get_dense()
    in_maps = _prep_dense_inputs(np.asarray(det_boxes), np.asarray(det_labels),
                                 np.asarray(boxes), np.asarray(labels))
    res = run_bass_kernel_spmd(nc, in_maps, list(range(N_CORES)))
    tot = np.zeros(2, dtype=np.float64)
    for c in range(N_CORES):
        tot += res.results[c]["partial"][0].astype(np.float64)
    loss = np.float32(np.float32(tot[0]) / np.float32(tot[1]))
    return np.asarray(loss, dtype=np.float32)


# ---------------------------------------------------------------------------
# dev helpers (not used by the grading harness)

def _numpy_shard_ref(in_map):
    """Reference for one core's partial, straight from the sharded layout."""
    detp = in_map["detp"]  # [128, 5, T]
    objr = in_map["objr"]  # [5, F]
    det = detp.transpose(2, 0, 1).reshape(-1, 5)  # [DET_PAD, 5]
    dx1, dy1, dx2, dy2, dlab = det.T
    ox1, oy1, ox2, oy2, olab = objr
    ad = (dx2 - dx1) * (dy2 - dy1)
    ao = (ox2 - ox1) * (oy2 - oy1)
    wx = np.maximum(np.minimum(ox2[None], dx2[:, None])
                    - np.maximum(ox1[None], dx1[:, None]), 0)
    wy = np.maximum(np.minimum(oy2[None], dy2[:, None])
                    - np.maximum(oy1[None], dy1[:, None]), 0)
    inter = wx * wy
    denom = ao[None] + ad[:, None] - inter
    iou = inter / denom
    eq = (olab[None] == dlab[:, None]).astype(np.float32)
    miou = iou * eq
    best = miou.max(axis=0)
    hmv = eq.max(axis=0)
    return np.array([np.sum((1 - best) * hmv), np.sum(hmv)], dtype=np.float32)


def _selftest_sim():
    from concourse.bass_interp import CoreSim
    rng = np.random.default_rng(0)
    det_boxes = rng.random((N_DET, 4), dtype=np.float32)
    det_boxes[:, 2:] += det_boxes[:, :2]
    boxes = rng.random((N_OBJ, 4), dtype=np.float32)
    boxes[:, 2:] += boxes[:, :2]
    det_labels = rng.integers(0, 21, N_DET)
    labels = rng.integers(0, 21, N_OBJ)
    in_maps = _prep_dense_inputs(det_boxes, det_labels, boxes, labels)

    nc = _get_dense()
    sim = CoreSim(nc)
    for k, v in in_maps[0].items():
        sim.tensor(k)[:] = v
    sim.simulate()
    got = sim.tensor("partial")[0]
    want = _numpy_shard_ref(in_maps[0])
    print("sim partial:", got, "numpy:", want,
          "relerr:", np.abs(got - want) / np.abs(want))


if __name__ == "__main__":
    _selftest_sim()


# revision 7
# speedup vs baseline: 11.8618x; 11.8618x over previous
"""Trainium2 Bass kernel for nn_AttackLoss (nms_detection).

Computes, for O=2048 ground-truth boxes vs D=8732 detections:
    best[o]  = max IoU over same-label detections
    loss     = sum(has_match * (1 - best)) / sum(has_match)

Sharding: objects are split across 8 NeuronCores (256 per core). Each core
holds all detections laid out on SBUF partitions (128 x 69 tiles) and its
object slab broadcast along the free axis; it reduces to two partial scalars
(sum of matched (1-best), match count). The host sums the 8 partials.
"""

import os
import tempfile
from contextlib import ExitStack

import numpy as np

import concourse.bacc as bacc
import concourse.bass as bass
import concourse.mybir as mybir
import concourse.tile as tile
from concourse.bass_isa import ReduceOp
from concourse.bass_utils import run_bass_kernel_spmd

F32 = mybir.dt.float32
OP = mybir.AluOpType
AX = mybir.AxisListType

N_CORES = 8
N_DET = 8732
N_OBJ = 2048
N_CLASSES = 21
OBJ_PER_CORE = N_OBJ // N_CORES  # 256
T_DET = 69                        # ceil(8732/128)
DET_PAD = 128 * T_DET             # 8832

# bucketed kernel static capacity
S_SLOTS = 3      # class-slots per core
FD = 512         # det capacity per slot
MAX_SLOTS = N_CORES * S_SLOTS


# ---------------------------------------------------------------------------
# custom DVE ops

_OPS_REGISTERED = {}


def _register_custom_ops():
    """Register fused DVE ops (official extension point: dve_ops.OPS)."""
    if _OPS_REGISTERED:
        return _OPS_REGISTERED
    import concourse.dve_ops as dve_ops
    from concourse.dve_spec import (Spec, Src0, Src1, C0, C1, relu, maxx,
                                    minn, lower)
    from concourse.dve_uop import DveOpSpec

    def make(name, spec, subdim=False):
        if name in dve_ops._SUB_OPCODE_FOR_NAME:
            for op in dve_ops.OPS:
                if op.name == name:
                    return op
        row = dve_ops._CUSTOM_DVE_ROW_BASE + len(dve_ops.OPS)
        assert row < 0x20
        shas = {}
        from concourse.dve_spec import _has_src1
        for ver in ("v3", "v4"):
            uops = lower(spec, ver=ver)
            shas[ver] = DveOpSpec(name=name, opcode=row, uops=uops,
                                  rd1_en=_has_src1(spec)).sha(ver)
        op = dve_ops.DveOp(name, spec, subdim, shas)
        dve_ops.OPS.append(op)
        dve_ops.CUSTOM_DVE_SPECS[name] = spec
        dve_ops._SUB_OPCODE_FOR_NAME[name] = row
        return op

    def _wx_ref(in0, in1, s0, s1, imm2):
        return np.maximum(
            np.minimum(in0.astype(np.float32), s0)
            - np.maximum(in1.astype(np.float32), s1), 0.0)

    # wx = relu(min(d_hi, o_hi) - max(d_lo, o_lo))
    wx_op = make("IOU_WX_ANT", Spec(
        body=relu(minn(Src0, C0) - maxx(Src1, C1)),
        reference=_wx_ref,
    ))

    def _t3_ref(in0, in1, s0, s1, imm2):
        return (s0 - in0.astype(np.float32) * in1).astype(np.float32)

    # t3 = area_o - wx*wy  (= area_o - inter)
    t3_op = make("IOU_T3_ANT", Spec(
        body=C0 - Src0 * Src1,
        reference=_t3_ref,
    ))

    def _ioumax_ref(in0, in1, s0, s1, imm2):
        b = ((s0 - in0.astype(np.float32)) * in1).astype(np.float32)
        b2 = b.reshape(b.shape[0], -1)
        seed = np.asarray(s1, np.float32).reshape(-1, 1) if isinstance(
            s1, np.ndarray) else np.full((b2.shape[0], 1), s1, np.float32)
        return b, np.maximum(b2.max(axis=-1, keepdims=True), seed)

    # iou = (area_o - t3) * recip ; accum_out = max(iou) over free dim
    ioumax_op = make("IOU_MAX_ANT", Spec(
        body=(C0 - Src0) * Src1,
        accum=maxx,
        accum_init=C1,
        reference=_ioumax_ref,
    ))

    _OPS_REGISTERED.update(wx=wx_op, t3=t3_op, ioumax=ioumax_op)
    return _OPS_REGISTERED


def _build_dense():
    """Dense kernel: all dets (on partitions) x this core's objects (free)."""
    nc = bacc.Bacc("TRN2", target_bir_lowering=False, debug=False,
                   num_devices=N_CORES)
    F = OBJ_PER_CORE

    detp_d = nc.dram_tensor("detp", [128, 5, T_DET], F32, kind="ExternalInput")
    objr_d = nc.dram_tensor("objr", [5, F], F32, kind="ExternalInput")
    part_d = nc.dram_tensor("partial", [1, 2], F32, kind="ExternalOutput")

    with tile.TileContext(nc) as tc, ExitStack() as ctx:
        cpool = ctx.enter_context(tc.tile_pool(name="const", bufs=1))
        wpool = ctx.enter_context(tc.tile_pool(name="work", bufs=3))

        detp = cpool.tile([128, 5, T_DET], F32)
        nc.sync.dma_start(detp[:], detp_d[:])
        # broadcast object rows across partitions
        names = ["ox1", "oy1", "ox2", "oy2", "olab"]
        ob = {}
        for i, nm in enumerate(names):
            row = cpool.tile([1, F], F32, tag=f"r_{nm}")
            nc.sync.dma_start(row[:], objr_d[i:i + 1, :])
            t = cpool.tile([128, F], F32, tag=f"b_{nm}")
            nc.gpsimd.partition_broadcast(t[:], row[:], channels=128)
            ob[nm] = t

        # object areas [128, F]
        aob = cpool.tile([128, F], F32)
        wob = wpool.tile([128, F], F32, tag="wob")
        nc.vector.tensor_tensor(wob[:], ob["ox2"][:], ob["ox1"][:], OP.subtract)
        hob = wpool.tile([128, F], F32, tag="hob")
        nc.vector.tensor_tensor(hob[:], ob["oy2"][:], ob["oy1"][:], OP.subtract)
        nc.vector.tensor_tensor(aob[:], wob[:], hob[:], OP.mult)

        # det areas [128, T]
        ad = cpool.tile([128, T_DET], F32)
        wd = wpool.tile([128, T_DET], F32, tag="wd")
        nc.vector.tensor_tensor(wd[:], detp[:, 2, :], detp[:, 0, :], OP.subtract)
        hd = wpool.tile([128, T_DET], F32, tag="hd")
        nc.vector.tensor_tensor(hd[:], detp[:, 3, :], detp[:, 1, :], OP.subtract)
        nc.vector.tensor_tensor(ad[:], wd[:], hd[:], OP.mult)

        bmax = cpool.tile([128, F], F32)
        nc.vector.memset(bmax[:], 0.0)
        hm = cpool.tile([128, F], F32)
        nc.vector.memset(hm[:], 0.0)

        for t in range(T_DET):
            dx1 = detp[:, 0, t:t + 1]
            dy1 = detp[:, 1, t:t + 1]
            dx2 = detp[:, 2, t:t + 1]
            dy2 = detp[:, 3, t:t + 1]
            dlab = detp[:, 4, t:t + 1]
            adt = ad[:, t:t + 1]

            mnx = wpool.tile([128, F], F32, tag="mnx")
            nc.vector.tensor_scalar(mnx[:], ob["ox2"][:], dx2, None, op0=OP.min)
            mxx = wpool.tile([128, F], F32, tag="mxx")
            nc.vector.tensor_scalar(mxx[:], ob["ox1"][:], dx1, None, op0=OP.max)
            wx = wpool.tile([128, F], F32, tag="wx")
            nc.vector.tensor_tensor(wx[:], mnx[:], mxx[:], OP.subtract)
            wxr = wpool.tile([128, F], F32, tag="wxr")
            nc.vector.tensor_scalar(wxr[:], wx[:], 0.0, None, op0=OP.max)

            mny = wpool.tile([128, F], F32, tag="mny")
            nc.vector.tensor_scalar(mny[:], ob["oy2"][:], dy2, None, op0=OP.min)
            mxy = wpool.tile([128, F], F32, tag="mxy")
            nc.vector.tensor_scalar(mxy[:], ob["oy1"][:], dy1, None, op0=OP.max)
            wy = wpool.tile([128, F], F32, tag="wy")
            nc.vector.tensor_tensor(wy[:], mny[:], mxy[:], OP.subtract)
            wyr = wpool.tile([128, F], F32, tag="wyr")
            nc.vector.tensor_scalar(wyr[:], wy[:], 0.0, None, op0=OP.max)

            inter = wpool.tile([128, F], F32, tag="inter")
            nc.vector.tensor_tensor(inter[:], wxr[:], wyr[:], OP.mult)
            sab = wpool.tile([128, F], F32, tag="sab")
            nc.vector.tensor_scalar(sab[:], aob[:], adt, None, op0=OP.add)
            denom = wpool.tile([128, F], F32, tag="denom")
            nc.vector.tensor_tensor(denom[:], sab[:], inter[:], OP.subtract)
            rec = wpool.tile([128, F], F32, tag="rec")
            nc.vector.reciprocal(rec[:], denom[:])
            iou = wpool.tile([128, F], F32, tag="iou")
            nc.vector.tensor_tensor(iou[:], inter[:], rec[:], OP.mult)

            eq = wpool.tile([128, F], F32, tag="eq")
            nc.vector.tensor_scalar(eq[:], ob["olab"][:], dlab, None,
                                    op0=OP.is_equal)
            miou = wpool.tile([128, F], F32, tag="miou")
            nc.vector.tensor_tensor(miou[:], iou[:], eq[:], OP.mult)

            nc.vector.tensor_tensor(bmax[:], bmax[:], miou[:], OP.max)
            nc.vector.tensor_tensor(hm[:], hm[:], eq[:], OP.max)

        bred = cpool.tile([128, F], F32)
        nc.gpsimd.partition_all_reduce(bred[:], bmax[:], 128, ReduceOp.max)
        hred = cpool.tile([128, F], F32)
        nc.gpsimd.partition_all_reduce(hred[:], hm[:], 128, ReduceOp.max)

        c1 = wpool.tile([1, F], F32, tag="c1")
        nc.vector.tensor_scalar(c1[:], bred[0:1, :], -1.0, 1.0,
                                op0=OP.mult, op1=OP.add)
        c2 = wpool.tile([1, F], F32, tag="c2")
        nc.vector.tensor_tensor(c2[:], c1[:], hred[0:1, :], OP.mult)

        outt = wpool.tile([1, 2], F32, tag="outt")
        nc.vector.tensor_reduce(outt[:, 0:1], c2[:], AX.X, OP.add)
        nc.vector.tensor_reduce(outt[:, 1:2], hred[0:1, :], AX.X, OP.add)
        nc.sync.dma_start(part_d[:], outt[:])

    nc.compile()
    return nc


def _build_bucket(fast_recip=True):
    """Class-bucketed kernel: each core runs S_SLOTS single-class slots.

    A slot is (<=128 objects of one class on partitions) x (<=FD dets of the
    same class on the free axis). No label masking needed inside a slot.
    """
    ops = _register_custom_ops()
    from concourse.dve_ops import RECIPROCAL_APPROX_FAST, RECIP_APPROX_FAST_CONSTS

    nc = bacc.Bacc("TRN2", target_bir_lowering=False, debug=False,
                   num_devices=N_CORES)

    detr_d = nc.dram_tensor("detr", [4 * S_SLOTS, FD], F32,
                            kind="ExternalInput")
    objs_d = nc.dram_tensor("objs", [128, S_SLOTS, 5], F32,
                            kind="ExternalInput")
    part_d = nc.dram_tensor("partial", [1, 2], F32, kind="ExternalOutput")

    with tile.TileContext(nc) as tc, ExitStack() as ctx:
        cpool = ctx.enter_context(tc.tile_pool(name="const", bufs=1))
        wpool = ctx.enter_context(tc.tile_pool(name="work", bufs=2))

        objs = cpool.tile([128, S_SLOTS, 5], F32)
        nc.sync.dma_start(objs[:], objs_d[:])

        acc = cpool.tile([128, 2], F32)
        nc.vector.memset(acc[:], 0.0)

        for s in range(S_SLOTS):
            bc = []
            for k in range(4):
                row = wpool.tile([1, FD], F32, tag=f"row{k}")
                nc.sync.dma_start(row[:], detr_d[4 * s + k:4 * s + k + 1, :])
                b = wpool.tile([128, FD], F32, tag=f"bc{k}")
                nc.gpsimd.partition_broadcast(b[:], row[:], channels=128)
                bc.append(b)
            dx1b, dy1b, dx2b, dy2b = bc

            wdt = wpool.tile([128, FD], F32, tag="wdt")
            nc.vector.tensor_tensor(wdt[:], dx2b[:], dx1b[:], OP.subtract)
            hdt = wpool.tile([128, FD], F32, tag="hdt")
            nc.vector.tensor_tensor(hdt[:], dy2b[:], dy1b[:], OP.subtract)
            adb = wpool.tile([128, FD], F32, tag="adb")
            nc.vector.tensor_tensor(adb[:], wdt[:], hdt[:], OP.mult)

            ox1 = objs[:, s, 0:1]
            oy1 = objs[:, s, 1:2]
            ox2 = objs[:, s, 2:3]
            oy2 = objs[:, s, 3:4]
            vmask = objs[:, s, 4:5]

            wo = wpool.tile([128, 1], F32, tag="wo")
            nc.vector.tensor_tensor(wo[:], ox2, ox1, OP.subtract)
            ho = wpool.tile([128, 1], F32, tag="ho")
            nc.vector.tensor_tensor(ho[:], oy2, oy1, OP.subtract)
            ao = wpool.tile([128, 1], F32, tag="ao")
            nc.vector.tensor_tensor(ao[:], wo[:], ho[:], OP.mult)

            wx = wpool.tile([128, FD], F32, tag="wx")
            nc.vector._custom_dve(ops["wx"], out=wx[:], in0=dx2b[:],
                                  in1=dx1b[:], s0=ox2, s1=ox1)
            wy = wpool.tile([128, FD], F32, tag="wy")
            nc.vector._custom_dve(ops["wx"], out=wy[:], in0=dy2b[:],
                                  in1=dy1b[:], s0=oy2, s1=oy1)
            t3 = wpool.tile([128, FD], F32, tag="t3")
            nc.vector._custom_dve(ops["t3"], out=t3[:], in0=wx[:], in1=wy[:],
                                  s0=ao[:])
            denom = wpool.tile([128, FD], F32, tag="denom")
            nc.vector.tensor_tensor(denom[:], t3[:], adb[:], OP.add)
            rec = wpool.tile([128, FD], F32, tag="rec")
            if fast_recip:
                nc.vector._custom_dve(RECIPROCAL_APPROX_FAST, out=rec[:],
                                      in0=denom[:],
                                      **RECIP_APPROX_FAST_CONSTS)
            else:
                nc.vector.reciprocal(rec[:], denom[:])

            scratch = wpool.tile([128, FD], F32, tag="scratch")
            best = wpool.tile([128, 1], F32, tag="best")
            nc.vector._custom_dve(ops["ioumax"], out=scratch[:],
                                  accum_out=best[:], in0=t3[:], in1=rec[:],
                                  s0=ao[:], s1=0.0)

            # slot_has_dets: real dets have x2 >= 0, pads are -5
            dmx = wpool.tile([128, 1], F32, tag="dmx")
            nc.vector.tensor_reduce(dmx[:], dx2b[:], AX.X, OP.max)
            sh = wpool.tile([128, 1], F32, tag="sh")
            nc.vector.tensor_scalar(sh[:], dmx[:], 0.0, None, op0=OP.is_ge)
            veff = wpool.tile([128, 1], F32, tag="veff")
            nc.vector.tensor_tensor(veff[:], vmask, sh[:], OP.mult)

            c1 = wpool.tile([128, 1], F32, tag="c1")
            nc.vector.tensor_scalar(c1[:], best[:], -1.0, 1.0,
                                    op0=OP.mult, op1=OP.add)
            cm = wpool.tile([128, 1], F32, tag="cm")
            nc.vector.tensor_tensor(cm[:], c1[:], veff[:], OP.mult)
            nc.vector.tensor_tensor(acc[:, 0:1], acc[:, 0:1], cm[:], OP.add)
            nc.vector.tensor_tensor(acc[:, 1:2], acc[:, 1:2], veff[:], OP.add)

        red = cpool.tile([128, 2], F32)
        nc.gpsimd.partition_all_reduce(red[:], acc[:], 128, ReduceOp.add)
        nc.sync.dma_start(part_d[:], red[0:1, :])

    nc.compile()
    return nc


def _prep_bucket_inputs(det_boxes, det_labels, boxes, labels):
    """Build per-core in_maps for the bucketed kernel, or None if the
    static capacity (S_SLOTS per core, FD dets / 128 objects per class)
    doesn't fit this input."""
    det_boxes = det_boxes.astype(np.float32)
    boxes = boxes.astype(np.float32)
    det_labels = np.asarray(det_labels)
    labels = np.asarray(labels)

    dc = np.bincount(det_labels, minlength=N_CLASSES)
    oc = np.bincount(labels, minlength=N_CLASSES)
    if dc.max() > FD or oc.max() > 128 or N_CLASSES > MAX_SLOTS:
        return None

    det_order = np.argsort(det_labels, kind="stable")
    obj_order = np.argsort(labels, kind="stable")
    det_off = np.concatenate([[0], np.cumsum(dc)])
    obj_off = np.concatenate([[0], np.cumsum(oc)])

    in_maps = []
    for c in range(N_CORES):
        detr = np.full((4 * S_SLOTS, FD), -5.0, dtype=np.float32)
        objs = np.zeros((128, S_SLOTS, 5), dtype=np.float32)
        objs[:, :, 0] = -9.0
        objs[:, :, 1] = -9.0
        objs[:, :, 2] = -8.0
        objs[:, :, 3] = -8.0
        for s in range(S_SLOTS):
            cls = c * S_SLOTS + s if c * S_SLOTS + s < N_CLASSES else None
            if cls is None:
                continue
            dsel = det_order[det_off[cls]:det_off[cls + 1]]
            osel = obj_order[obj_off[cls]:obj_off[cls + 1]]
            nd, no = len(dsel), len(osel)
            detr[4 * s + 0, :nd] = det_boxes[dsel, 0]
            detr[4 * s + 1, :nd] = det_boxes[dsel, 1]
            detr[4 * s + 2, :nd] = det_boxes[dsel, 2]
            detr[4 * s + 3, :nd] = det_boxes[dsel, 3]
            objs[:no, s, 0:4] = boxes[osel]
            objs[:no, s, 4] = 1.0
        in_maps.append({"detr": detr, "objs": objs})
    return in_maps


def _prep_dense_inputs(det_boxes, det_labels, boxes, labels):
    """Build per-core in_maps for the dense kernel (numpy only)."""
    det = np.full((DET_PAD, 5), -5.0, dtype=np.float32)
    det[:N_DET, 0:4] = det_boxes.astype(np.float32)
    det[:N_DET, 4] = det_labels.astype(np.float32)
    det[N_DET:, 4] = -1.0
    # [DET_PAD, 5] -> [T, 128, 5] -> [128, 5, T]
    detp = np.ascontiguousarray(
        det.reshape(T_DET, 128, 5).transpose(1, 2, 0))

    in_maps = []
    for c in range(N_CORES):
        sl = slice(c * OBJ_PER_CORE, (c + 1) * OBJ_PER_CORE)
        objr = np.empty((5, OBJ_PER_CORE), dtype=np.float32)
        objr[0:4, :] = boxes[sl].astype(np.float32).T
        objr[4, :] = labels[sl].astype(np.float32)
        in_maps.append({"detp": detp, "objr": objr})
    return in_maps


_CACHE = {}


def _get_dense():
    if "dense" not in _CACHE:
        _CACHE["dense"] = _build_dense()
    return _CACHE["dense"]


def _get_bucket():
    if "bucket" not in _CACHE:
        _CACHE["bucket"] = _build_bucket()
    return _CACHE["bucket"]


def _run_partials(nc, in_maps):
    res = run_bass_kernel_spmd(nc, in_maps, list(range(N_CORES)))
    tot = np.zeros(2, dtype=np.float32)
    for c in range(N_CORES):
        tot += res.results[c]["partial"][0]
    return np.asarray(np.float32(tot[0] / tot[1]))


def kernel(det_boxes, det_scores, det_labels, boxes, labels):
    det_boxes = np.asarray(det_boxes)
    det_labels = np.asarray(det_labels)
    boxes = np.asarray(boxes)
    labels = np.asarray(labels)
    in_maps = _prep_bucket_inputs(det_boxes, det_labels, boxes, labels)
    if in_maps is not None:
        return _run_partials(_get_bucket(), in_maps)
    in_maps = _prep_dense_inputs(det_boxes, det_labels, boxes, labels)
    return _run_partials(_get_dense(), in_maps)


# ---------------------------------------------------------------------------
# dev helpers (not used by the grading harness)

def _numpy_shard_ref(in_map):
    """Reference for one core's partial, straight from the sharded layout."""
    detp = in_map["detp"]  # [128, 5, T]
    objr = in_map["objr"]  # [5, F]
    det = detp.transpose(2, 0, 1).reshape(-1, 5)  # [DET_PAD, 5]
    dx1, dy1, dx2, dy2, dlab = det.T
    ox1, oy1, ox2, oy2, olab = objr
    ad = (dx2 - dx1) * (dy2 - dy1)
    ao = (ox2 - ox1) * (oy2 - oy1)
    wx = np.maximum(np.minimum(ox2[None], dx2[:, None])
                    - np.maximum(ox1[None], dx1[:, None]), 0)
    wy = np.maximum(np.minimum(oy2[None], dy2[:, None])
                    - np.maximum(oy1[None], dy1[:, None]), 0)
    inter = wx * wy
    denom = ao[None] + ad[:, None] - inter
    iou = inter / denom
    eq = (olab[None] == dlab[:, None]).astype(np.float32)
    miou = iou * eq
    best = miou.max(axis=0)
    hmv = eq.max(axis=0)
    return np.array([np.sum((1 - best) * hmv), np.sum(hmv)], dtype=np.float32)


def _full_numpy_ref(det_boxes, det_labels, boxes, labels):
    ov_all = []
    for c0 in range(0, N_OBJ, 256):
        b = boxes[c0:c0 + 256].astype(np.float64)
        d = det_boxes.astype(np.float64)
        lo = np.maximum(b[:, None, :2], d[None, :, :2])
        hi = np.minimum(b[:, None, 2:], d[None, :, 2:])
        wh = np.clip(hi - lo, 0, None)
        inter = wh[..., 0] * wh[..., 1]
        ao = (b[:, 2] - b[:, 0]) * (b[:, 3] - b[:, 1])
        ad = (d[:, 2] - d[:, 0]) * (d[:, 3] - d[:, 1])
        union = ao[:, None] + ad[None, :] - inter
        iou = inter / union
        same = labels[c0:c0 + 256, None] == det_labels[None, :]
        masked = np.where(same, iou, -np.inf)
        ov_all.append((masked.max(axis=1), same.any(axis=1)))
    best = np.concatenate([x[0] for x in ov_all])
    hmv = np.concatenate([x[1] for x in ov_all])
    npos = hmv.sum()
    return np.float32(np.sum(np.where(hmv, 1.0 - best, 0.0)) / npos)


def _rand_inputs(seed=0):
    rng = np.random.default_rng(seed)
    def mk(n):
        cxy = rng.random((n, 2), dtype=np.float32)
        wh = rng.random((n, 2), dtype=np.float32) * 0.3 + 0.02
        lo = np.clip(cxy - wh / 2, 0, 1)
        hi = np.clip(cxy + wh / 2, 0, 1)
        return np.concatenate([lo, hi], axis=1)
    return (mk(N_DET), rng.integers(0, 21, N_DET),
            mk(N_OBJ), rng.integers(0, 21, N_OBJ))


def _sim_core(nc, in_map, out_name="partial"):
    from concourse.bass_interp import CoreSim
    sim = CoreSim(nc)
    for k, v in in_map.items():
        sim.tensor(k)[:] = v
    sim.simulate()
    return np.array(sim.tensor(out_name))


def _selftest_sim():
    det_boxes, det_labels, boxes, labels = _rand_inputs(0)
    want_loss = _full_numpy_ref(det_boxes, det_labels, boxes, labels)

    # bucketed: simulate every core, combine
    in_maps = _prep_bucket_inputs(det_boxes, det_labels, boxes, labels)
    assert in_maps is not None
    nc = _get_bucket()
    tot = np.zeros(2, dtype=np.float32)
    for c in range(N_CORES):
        p = _sim_core(nc, in_maps[c])[0]
        tot += p
    got = np.float32(tot[0] / tot[1])
    print(f"bucket sim loss: {got}  numpy ref: {want_loss}  "
          f"relerr: {abs(got - want_loss) / abs(want_loss):.3e}")


if __name__ == "__main__":
    _selftest_sim()
